# revision 1
# baseline (speedup 1.0000x reference)
"""Trainium2 Bass kernel for nn_BaseKernelSetConv (gnn_message_passing).

Strategy (8 NeuronCores, data-parallel over nodes):
  - Focal scores: computed DENSELY. Node i appears as a focal exactly once
    (the per-degree selected_index sets partition all nodes), so each core
    streams its contiguous 125k-node shard of x, normalizes rows on-chip,
    transposes 128-node blocks on the PE, and matmuls against all four
    focal kernel sets at once -> (64, shard) scores. The host later keeps
    only the 16-wide band matching each node's actual degree.
  - Neighbor scores: per (core, degree) the neighbor rows are gathered with
    [128,1]-form indirect DMAs (one row per partition per instruction - the
    only indirect form the SWDGE ucode implements correctly), normalized,
    transposed per 128-node block and matmuled against the stacked
    (unit-normalized, /deg) neighbor kernels -> (16, n_d) scores.
  - Host assembles: res[node, band(deg)] = focal_band + neighbor_scores.
"""

import sys
import numpy as np

sys.path.insert(0, "/opt/trn_rl_repo")

F = 32
K = 16
NCORES = 8

_PROG = None


def _chunks(total_nodes, g):
    """Split total_nodes (multiple of 128) into (start, G) chunks of
    128*G nodes with a possibly smaller tail."""
    out = []
    start = 0
    nb = total_nodes // 128
    while start < nb:
        gc = min(g, nb - start)
        out.append((start * 128, gc))
        start += gc
    return out


def _configure(n, lshard, npad, gf, ndc, gsup):
    global N, SHARD, LSHARD, NPAD, GF, NDC, GSUP, FOCAL_CHUNKS, NEI_CHUNKS, _PROG
    N = n
    SHARD = n // NCORES
    LSHARD = lshard                   # dense shard padded to mult of 128
    NPAD = npad                       # padded table rows
    GF = gf                           # focal: nodes per partition per chunk
    NDC = dict(ndc)                   # padded nodes per (core, degree)
    GSUP = dict(gsup)                 # neighbor: nodes/partition/supertile
    FOCAL_CHUNKS = _chunks(LSHARD, GF)
    NEI_CHUNKS = {d: _chunks(NDC[d], GSUP[d]) for d in (1, 2, 3, 4)}
    _PROG = None


_configure(1_000_000, 125056, 1000576, 32,
           {1: 25728, 2: 38016, 3: 38016, 4: 25728},
           {1: 48, 2: 24, 3: 16, 4: 12})


def _set_ndc(ndc):
    """Tighten padded per-(core,degree) node counts to the actual input
    (one gather instruction per 128 wasted pad rows otherwise)."""
    global NDC, NEI_CHUNKS, _PROG
    if dict(ndc) != NDC:
        NDC = dict(ndc)
        NEI_CHUNKS = {d: _chunks(NDC[d], GSUP[d]) for d in (1, 2, 3, 4)}
        _PROG = None


def _build_program():
    import concourse.bass as bass
    import concourse.tile as tile
    from concourse import bacc, mybir
    from concourse.masks import make_identity

    f32 = mybir.dt.float32
    i32 = mybir.dt.int32
    AX = mybir.AxisListType.X
    MUL = mybir.AluOpType.mult
    ADD = mybir.AluOpType.add

    nc = bacc.Bacc("TRN2", target_bir_lowering=False, debug=False,
                   num_devices=NCORES)
    x_d = nc.dram_tensor("x", (NPAD, F), f32, kind="ExternalInput").ap()
    xsh_d = nc.dram_tensor("xsh", (LSHARD, F), f32, kind="ExternalInput").ap()
    wf_d = nc.dram_tensor("wf", (F, 64), f32, kind="ExternalInput").ap()
    wn_d = {d: nc.dram_tensor(f"wn{d}", (d * F, K), f32,
                              kind="ExternalInput").ap() for d in (1, 2, 3, 4)}
    idx_d = {d: nc.dram_tensor(f"idx{d}", (NDC[d] * d,), i32,
                               kind="ExternalInput").ap() for d in (1, 2, 3, 4)}
    focal_o = nc.dram_tensor("focal_o", (64, LSHARD), f32,
                             kind="ExternalOutput").ap()
    nei_o = {d: nc.dram_tensor(f"nei_o{d}", (K, NDC[d]), f32,
                               kind="ExternalOutput").ap() for d in (1, 2, 3, 4)}

    with tile.TileContext(nc) as tc:
        with tc.tile_pool(name="wp", bufs=1) as wp, \
             tc.tile_pool(name="stage", bufs=3) as stage_p, \
             tc.tile_pool(name="scr", bufs=2) as scr_p, \
             tc.tile_pool(name="nrm", bufs=2) as nrm_p, \
             tc.tile_pool(name="idxp", bufs=3) as idx_p, \
             tc.tile_pool(name="tsb", bufs=3) as tsb_p, \
             tc.tile_pool(name="ost", bufs=2) as ost_p, \
             tc.tile_pool(name="fstage", bufs=2) as fstage_p, \
             tc.tile_pool(name="fscr", bufs=2) as fscr_p, \
             tc.tile_pool(name="fnrm", bufs=2) as fnrm_p, \
             tc.tile_pool(name="ftsb", bufs=3) as ftsb_p, \
             tc.tile_pool(name="fost", bufs=2) as fost_p, \
             tc.tile_pool(name="tps", bufs=2, space="PSUM") as tps_p, \
             tc.tile_pool(name="ftps", bufs=2, space="PSUM") as ftps_p, \
             tc.tile_pool(name="fps", bufs=2, space="PSUM") as fps_p, \
             tc.tile_pool(name="sps", bufs=2, space="PSUM") as sps_p:

            ident = wp.tile([128, 128], f32)
            make_identity(nc, ident[:])
            wf_sb = wp.tile([F, 64], f32, tag="wf")
            nc.sync.dma_start(wf_sb[:], wf_d[:])
            wn_sb = {}
            for d in (1, 2, 3, 4):
                wn_sb[d] = wp.tile([d * F, K], f32, tag=f"wn{d}",
                                   name=f"wn_sb{d}")
                nc.sync.dma_start(wn_sb[d][:], wn_d[d][:])

            def normalize(st, rows, scrp, nrmp, pfx):
                """st: [128, rows*F] raw rows -> returns [128, rows*F]
                normalized tile (unit L2 per 32-float row)."""
                scr = scrp.tile([128, rows * F], f32, tag=pfx + "scr",
                                name=pfx + "scr")
                nc.scalar.square(scr[:], st[:])
                n2 = nrmp.tile([128, rows], f32, tag=pfx + "n2",
                               name=pfx + "n2")
                nc.vector.tensor_reduce(
                    n2[:], scr[:].rearrange("p (r f) -> p r f", f=F),
                    axis=AX, op=ADD)
                r2 = nrmp.tile([128, rows], f32, tag=pfx + "r2",
                               name=pfx + "r2")
                nc.vector.reciprocal(r2[:], n2[:])
                inv = nrmp.tile([128, rows], f32, tag=pfx + "inv",
                                name=pfx + "inv")
                nc.scalar.sqrt(inv[:], r2[:])
                nc.vector.tensor_tensor(
                    out=scr[:].rearrange("p (r f) -> p r f", f=F),
                    in0=st[:].rearrange("p (r f) -> p r f", f=F),
                    in1=inv[:].rearrange("p (r u) -> p r u", u=1)
                        .to_broadcast([128, rows, F]),
                    op=MUL)
                return scr

            focal_cols = {}
            c = 0
            for start, gc in FOCAL_CHUNKS:
                focal_cols[start] = c
                c += gc * 128
            nei_cols = {}
            for d in (1, 2, 3, 4):
                c = 0
                for start, gs in NEI_CHUNKS[d]:
                    nei_cols[(d, start)] = c
                    c += gs * 128

            def emit_focal(start, gc):
                col = focal_cols[start]
                st = fstage_p.tile([128, gc * F], f32, tag="fstage",
                                   name="fstage")
                nc.sync.dma_start(
                    st[:],
                    xsh_d[start:start + 128 * gc, :]
                        .rearrange("(p g) f -> p (g f)", p=128))
                nrmed = normalize(st, gc, fscr_p, fnrm_p, "f")
                ost = fost_p.tile([64, gc * 128], f32, tag="fost", name="fost")
                for g in range(gc):
                    tp = ftps_p.tile([128, 128], f32, tag="ftps", name="ftps")
                    nc.tensor.transpose(
                        out=tp[:F, :], in_=nrmed[:, g * F:(g + 1) * F],
                        identity=ident[:])
                    ts = ftsb_p.tile([128, 128], f32, tag="ftsb", name="ftsb")
                    nc.any.tensor_copy(ts[:F, :], tp[:F, :])
                    fp = fps_p.tile([64, 128], f32, tag="fps", name="fps")
                    nc.tensor.matmul(fp[:], lhsT=wf_sb[:], rhs=ts[:F, :],
                                     start=True, stop=True)
                    nc.any.tensor_copy(ost[:, g * 128:(g + 1) * 128], fp[:])
                nc.sync.dma_start(focal_o[:, col:col + gc * 128], ost[:])

            def emit_nei(d, start, gs):
                col = nei_cols[(d, start)]
                rows = gs * d
                it = idx_p.tile([128, rows], i32, tag="idx", name="idx")
                nc.sync.dma_start(
                    it[:],
                    idx_d[d][start * d:(start + 128 * gs) * d]
                        .rearrange("(p r) -> p r", p=128))
                st = stage_p.tile([128, rows * F], f32, tag="stage",
                                  name="stage")
                for r in range(rows):
                    nc.gpsimd.indirect_dma_start(
                        out=st[:, r * F:(r + 1) * F],
                        out_offset=None,
                        in_=x_d[:],
                        in_offset=bass.IndirectOffsetOnAxis(
                            ap=it[:, r:r + 1], axis=0),
                    )
                nrmed = normalize(st, rows, scr_p, nrm_p, "n")
                ost = ost_p.tile([K, gs * 128], f32, tag="ost", name="ost")
                for g in range(gs):
                    tp = tps_p.tile([128, 128], f32, tag="tps", name="tps")
                    nc.tensor.transpose(
                        out=tp[:d * F, :],
                        in_=nrmed[:, g * d * F:(g + 1) * d * F],
                        identity=ident[:])
                    ts = tsb_p.tile([128, 128], f32, tag="tsb", name="tsb")
                    nc.any.tensor_copy(ts[:d * F, :], tp[:d * F, :])
                    sp = sps_p.tile([K, 128], f32, tag="sps", name="sps")
                    nc.tensor.matmul(sp[:], lhsT=wn_sb[d][:],
                                     rhs=ts[:d * F, :],
                                     start=True, stop=True)
                    nc.any.tensor_copy(ost[:, g * 128:(g + 1) * 128], sp[:])
                nc.sync.dma_start(nei_o[d][:, col:col + gs * 128], ost[:])

            # Interleave: neighbor supertiles carry the critical path (Pool
            # descriptor generation); focal chunks slot into idle engines.
            # Small (tail) supertiles are emitted last so the post-final-
            # gather drain is as short as possible; focal chunks are front-
            # loaded to finish well inside the gather shadow.
            nei_items = [(d, s, g) for d in (1, 2, 3, 4)
                         for s, g in NEI_CHUNKS[d]]
            nei_items.sort(key=lambda t: -(t[2] * t[0]))   # big gathers first
            focal_items = list(FOCAL_CHUNKS)
            fi = 0
            pace = max(1, (len(nei_items) * 3) // 4)       # done by ~75% mark
            for i, (d, s, g) in enumerate(nei_items):
                emit_nei(d, s, g)
                while fi < len(focal_items) and fi + 1 <= (i + 1) * len(focal_items) // pace:
                    emit_focal(*focal_items[fi])
                    fi += 1
            while fi < len(focal_items):
                emit_focal(*focal_items[fi])
                fi += 1

    nc.compile()
    return nc


def _unit_rows(a):
    a = a.astype(np.float64)
    return (a / (np.linalg.norm(a, axis=-1, keepdims=True) + 1e-8)).astype(np.float32)


def host_prep(inputs):
    """Build per-core device inputs + bookkeeping for assembly."""
    x = np.ascontiguousarray(np.asarray(inputs["x"], dtype=np.float32))
    sels = {d: np.asarray(inputs[f"selected_index_deg{d}"]).astype(np.int64)
            for d in (1, 2, 3, 4)}
    neis = {d: np.asarray(inputs[f"nei_index_deg{d}"]).astype(np.int64)
            .reshape(-1, d) for d in (1, 2, 3, 4)}

    xpad = np.ones((NPAD, F), np.float32)   # pad rows finite (avoid 0-norm NaN)
    xpad[:N] = x

    deg = np.zeros(N, np.int8)
    pos = np.zeros(N, np.int64)
    for d in (1, 2, 3, 4):
        deg[sels[d]] = d
        pos[sels[d]] = np.arange(sels[d].shape[0])

    # weights
    wf_all = np.concatenate(
        [_unit_rows(np.asarray(inputs[f"W_focal{d}"], np.float32))
         for d in (1, 2, 3, 4)], axis=0)            # (64, 32)
    wf_lhsT = np.ascontiguousarray(wf_all.T)        # (32, 64)
    wn_lhsT = {}
    for d in (1, 2, 3, 4):
        wn = np.asarray(inputs[f"W_nei{d}"], np.float32)   # (16, d, 32)
        u = _unit_rows(wn.reshape(-1, F)).reshape(K, d, F) / d
        wn_lhsT[d] = np.ascontiguousarray(u.reshape(K, d * F).T)  # (d*32, 16)

    # tighten per-(core,degree) padding to the actual degree distribution
    all_nodes = {}
    maxcnt = {d: 0 for d in (1, 2, 3, 4)}
    for c in range(NCORES):
        lo, hi = c * SHARD, (c + 1) * SHARD
        shard_deg = deg[lo:hi]
        for d in (1, 2, 3, 4):
            nodes_cd = np.nonzero(shard_deg == d)[0] + lo   # ascending ids
            all_nodes[(c, d)] = nodes_cd
            maxcnt[d] = max(maxcnt[d], nodes_cd.shape[0])
    _set_ndc({d: ((maxcnt[d] + 127) // 128) * 128 for d in (1, 2, 3, 4)})

    in_maps = []
    book = []      # per core: {d: (nodes_cd, cnt)}
    for c in range(NCORES):
        lo, hi = c * SHARD, (c + 1) * SHARD
        xsh = xpad[lo:lo + LSHARD]
        m = {"x": xpad, "xsh": np.ascontiguousarray(xsh),
             "wf": wf_lhsT}
        bk = {}
        for d in (1, 2, 3, 4):
            m[f"wn{d}"] = wn_lhsT[d]
            nodes_cd = all_nodes[(c, d)]
            cnt = nodes_cd.shape[0]
            assert cnt <= NDC[d], (c, d, cnt)
            nei_cd = np.zeros((NDC[d], d), np.int32)
            nei_cd[:cnt] = neis[d][pos[nodes_cd]].astype(np.int32)
            # device layout: per supertile s (start,gs): [128, gs*d] with
            # (p, g*d+slot) = nei of node_local start + p*gs + g
            flat = np.empty(NDC[d] * d, np.int32)
            o = 0
            for start, gs in NEI_CHUNKS[d]:
                slab = nei_cd[start:start + 128 * gs].reshape(128, gs * d)
                flat[o:o + slab.size] = slab.reshape(-1)
                o += slab.size
            m[f"idx{d}"] = flat
            bk[d] = (nodes_cd, cnt)
        in_maps.append(m)
        book.append(bk)
    return in_maps, book


def _uninterleave(arr, chunks):
    """arr: (B, total_cols) device-order -> (total_nodes, B) node-local order.
    Device col order per chunk: g*128 + p ; node_local = start + p*gc + g."""
    b = arr.shape[0]
    total = sum(gc * 128 for _, gc in chunks)
    out = np.empty((total, b), arr.dtype)
    col = 0
    for start, gc in chunks:
        blk = arr[:, col:col + gc * 128].reshape(b, gc, 128)
        out[start:start + gc * 128] = blk.transpose(2, 1, 0).reshape(gc * 128, b)
        col += gc * 128
    return out


def assemble(results, book):
    res = np.zeros((N, 64), np.float32)
    for c in range(NCORES):
        lo = c * SHARD
        focal = _uninterleave(results[c]["focal_o"], FOCAL_CHUNKS)  # (LSHARD, 64)
        for d in (1, 2, 3, 4):
            nodes_cd, cnt = book[c][d]
            nei = _uninterleave(results[c][f"nei_o{d}"], NEI_CHUNKS[d])  # (NDC,16)
            band = slice(16 * (d - 1), 16 * d)
            res[nodes_cd, band] = focal[nodes_cd - lo, band] + nei[:cnt]
    return res


LAST_RESULTS = None


def kernel(**inputs):
    global _PROG, LAST_RESULTS
    import os
    from concourse.bass_utils import run_bass_kernel_spmd
    in_maps, book = host_prep(inputs)   # may retune NDC -> resets _PROG
    if _PROG is None:
        _PROG = _build_program()
    trace = bool(os.environ.get("BKC_TRACE"))
    res = run_bass_kernel_spmd(_PROG, in_maps, core_ids=list(range(NCORES)),
                               trace=trace)
    LAST_RESULTS = res
    return assemble(res.results, book)


# ---------------------------------------------------------------------------
# numpy emulation of the device program (for fast host-logic validation)
def _emulate_core(m):
    x = m["x"]
    out = {}
    xs = m["xsh"].astype(np.float64)
    y = xs / (np.sqrt((xs * xs).sum(-1, keepdims=True)))
    z = (y @ m["wf"].astype(np.float64))            # (LSHARD, 64)
    focal = np.empty((64, LSHARD), np.float32)
    col = 0
    for start, gc in FOCAL_CHUNKS:
        blk = z[start:start + 128 * gc].reshape(128, gc, 64)
        focal[:, col:col + gc * 128] = (
            blk.transpose(2, 1, 0).reshape(64, gc * 128))
        col += gc * 128
    out["focal_o"] = focal
    for d in (1, 2, 3, 4):
        flat = m[f"idx{d}"]
        nei_out = np.empty((K, NDC[d]), np.float32)
        col = 0
        o = 0
        for start, gs in NEI_CHUNKS[d]:
            slab = flat[o:o + 128 * gs * d].reshape(128, gs, d)
            o += 128 * gs * d
            g = x[slab].astype(np.float64)          # (128, gs, d, 32)
            g = g / np.sqrt((g * g).sum(-1, keepdims=True))
            sc = np.einsum("pgdf,dfk->kgp", g,
                           m[f"wn{d}"].astype(np.float64).reshape(d, F, K))
            nei_out[:, col:col + gs * 128] = sc.reshape(K, gs * 128)
            col += gs * 128
        out[f"nei_o{d}"] = nei_out
    return out


def kernel_emulated(**inputs):
    in_maps, book = host_prep(inputs)
    results = [_emulate_core(m) for m in in_maps]
    return assemble(results, book)



# revision 2
# speedup vs baseline: 1.1265x; 1.1265x over previous
"""Trainium2 Bass kernel v2 for nn_BaseKernelSetConv (gnn_message_passing).

Architecture (8 cores, data-parallel over focal nodes):
  - Host normalizes x (unit rows) and builds a 256-B-stride fp32 table
    tab[N, 64] (32 used + 32 pad floats per row).
  - Neighbor gather: per core, its ~312.5k edges are grouped by source
    window (32 windows of 31250 rows — int16 dma_gather index range) and
    by (degree, slot) in 128-edge-aligned segments.  Four SWDGE queues run
    InstDMAGatherAnt concurrently (~4.4 ns/idx measured vs ~11 serial).
    Dynamic per-chunk counts (register-loaded) skip the -1 padded tails.
  - Per 512-edge tile: DVE cast fp32->bf16 (strided, drops pad floats), PE
    transpose, then a matmul whose stationary is a host-streamed
    block-diagonal [128, 64] of the four chunk-cols' unit-normalized
    kernel weights (zero rows kill pad lanes) -> 16 scores per edge.
  - Focal: host pre-normalizes, degree-groups, and pre-transposes the
    shard (xshT); 64 big matmuls against block-diag focal weights.
  - Host assembly: per-(d,j) vectorized adds of edge scores into nodes
    (each node appears once per (d,j) -> no scatter collisions).
"""

import sys
import numpy as np
import ml_dtypes

sys.path.insert(0, "/opt/trn_rl_repo")

N = 1_000_000
F = 32
K = 16
NCORES = 8
SHARD = N // NCORES            # 125_000

WROWS = 31_250                 # table rows per gather window (int16-safe)
NWIN = N // WROWS              # 32
NI = 12_288                    # idxs per dma_gather chunk
NCHW = 1                       # chunks per window
NIW = NI * NCHW                # 12_288 padded edge slots per window
NCH = NWIN * NCHW              # 64 chunks per core
NTILE = NI // 512              # 12 transpose tiles per chunk
ESTREAM = NWIN * NIW           # 393_216 edge-stream slots per core

SEGF = [26_624, 38_912, 38_912, 26_624]   # focal per-degree segment pads
SHPAD = sum(SEGF)              # 131_072
FTILES = SHPAD // 2048         # 64 focal matmul tiles (512 cols each)
_SEGB = np.cumsum([0] + SEGF)  # segment boundaries in nodes

DJ = [(d, j) for d in (1, 2, 3, 4) for j in range(d)]   # 10 (degree,slot)

_PROG = None
LAST_RESULTS = None


def _build_program():
    import concourse.tile as tile
    from concourse import bacc, mybir
    from concourse.masks import make_identity

    f32 = mybir.dt.float32
    bf16 = mybir.dt.bfloat16
    i16 = mybir.dt.int16
    i32 = mybir.dt.int32

    nc = bacc.Bacc("TRN2", target_bir_lowering=False, debug=False,
                   num_devices=NCORES, num_swdge_queues=4)
    tab_d = nc.dram_tensor("tab", (N, 64), f32, kind="ExternalInput").ap()
    idx_d = nc.dram_tensor("idx", (NCH, 128, NI // 16), i16,
                           kind="ExternalInput").ap()
    cnt_d = nc.dram_tensor("cnt", (1, NCH), i32, kind="ExternalInput").ap()
    wt_d = nc.dram_tensor("wt", (NCH, 128, NTILE * 64), bf16,
                          kind="ExternalInput").ap()
    xshT_d = nc.dram_tensor("xshT", (128, SHPAD // 4), bf16,
                            kind="ExternalInput").ap()
    wf_d = nc.dram_tensor("wf", (128, 256), bf16, kind="ExternalInput").ap()
    nei_o = nc.dram_tensor("nei_o", (NCH, 64, NTILE * 128), bf16,
                           kind="ExternalOutput").ap()
    focal_o = nc.dram_tensor("focal_o", (FTILES, 64, 512), bf16,
                             kind="ExternalOutput").ap()

    fseg_of = []
    for ft in range(FTILES):
        lo = ft * 2048
        s = int(np.searchsorted(_SEGB, lo, side="right")) - 1
        fseg_of.append(s)

    with tile.TileContext(nc) as tc:
        with tc.tile_pool(name="wp", bufs=1) as wp, \
             tc.tile_pool(name="gt", bufs=3) as gt_p, \
             tc.tile_pool(name="gtb", bufs=3) as gtb_p, \
             tc.tile_pool(name="tsb", bufs=4) as tsb_p, \
             tc.tile_pool(name="idxp", bufs=3) as idx_p, \
             tc.tile_pool(name="wtp", bufs=3) as wt_p, \
             tc.tile_pool(name="ost", bufs=3) as ost_p, \
             tc.tile_pool(name="xtp", bufs=3) as xt_p, \
             tc.tile_pool(name="fob", bufs=3) as fo_p, \
             tc.tile_pool(name="tps", bufs=3, space="PSUM") as tps_p, \
             tc.tile_pool(name="mps", bufs=3, space="PSUM") as mps_p, \
             tc.tile_pool(name="fps", bufs=2, space="PSUM") as fps_p:

            ident = wp.tile([128, 128], bf16)
            make_identity(nc, ident[:])
            cnt_sb = wp.tile([1, NCH], i32, tag="cnt")
            nc.sync.dma_start(cnt_sb[:], cnt_d[:])
            wf_sb = wp.tile([128, 256], bf16, tag="wf")
            nc.sync.dma_start(wf_sb[:], wf_d[:])

            def emit_chunk(ch):
                w = ch // NCHW
                it = idx_p.tile([128, NI // 16], i16, tag="idx")
                nc.sync.dma_start(it[:], idx_d[ch])
                wt = wt_p.tile([128, NTILE * 64], bf16, tag="wt")
                nc.sync.dma_start(wt[:], wt_d[ch])
                gt = gt_p.tile([128, (NI // 128) * 64], f32, tag="gt")
                if ch < 3:
                    nc.vector.memset(gt[:], 0.0)
                tmp = nc.gpsimd.alloc_register(f"cnt{ch}")
                nc.gpsimd.reg_load(tmp, cnt_sb[0:1, ch:ch + 1])
                cntv = nc.gpsimd.snap(tmp, donate=True)
                nc.gpsimd.dma_gather(
                    out_ap=gt[:].rearrange("p (c e) -> p c e", e=64),
                    in_ap=tab_d[w * WROWS:(w + 1) * WROWS, :],
                    idxs_ap=it[:],
                    num_idxs=NI,
                    num_idxs_reg=cntv,
                    elem_size=64,
                    single_packet=False,
                    queue_num=ch % 4,
                )
                gtb = gtb_p.tile([128, (NI // 128) * 32], bf16, tag="gtb")
                nc.vector.tensor_copy(
                    gtb[:].rearrange("p (c f) -> p c f", f=F),
                    gt[:].rearrange("p (c e) -> p c e", e=64)[:, :, 0:F])
                ost = ost_p.tile([64, NTILE * 128], bf16, tag="ost")
                for t in range(NTILE):
                    tp = tps_p.tile([128, 128], bf16, tag="tp")
                    nc.tensor.transpose(
                        out=tp[:], in_=gtb[:, t * 128:(t + 1) * 128],
                        identity=ident[:])
                    ts = tsb_p.tile([128, 128], bf16, tag="ts")
                    nc.scalar.copy(ts[:], tp[:])
                    mp = mps_p.tile([64, 128], f32, tag="mp")
                    nc.tensor.matmul(mp[:], lhsT=wt[:, t * 64:(t + 1) * 64],
                                     rhs=ts[:], start=True, stop=True)
                    nc.vector.tensor_copy(ost[:, t * 128:(t + 1) * 128], mp[:])
                nc.sync.dma_start(nei_o[ch], ost[:])

            def emit_focal(ft):
                xt = xt_p.tile([128, 512], bf16, tag="xt")
                nc.sync.dma_start(xt[:], xshT_d[:, ft * 512:(ft + 1) * 512])
                s = fseg_of[ft]
                fp = fps_p.tile([64, 512], f32, tag="fp")
                nc.tensor.matmul(fp[:], lhsT=wf_sb[:, s * 64:(s + 1) * 64],
                                 rhs=xt[:], start=True, stop=True)
                fo = fo_p.tile([64, 512], bf16, tag="fo")
                nc.vector.tensor_copy(fo[:], fp[:])
                nc.sync.dma_start(focal_o[ft], fo[:])

            for ch in range(NCH):
                emit_chunk(ch)
                emit_focal(2 * ch)      # FTILES == 2 * NCH
                emit_focal(2 * ch + 1)

    nc.compile()
    return nc


def _unit_rows(a):
    a = a.astype(np.float64)
    return (a / (np.linalg.norm(a, axis=-1, keepdims=True) + 1e-8)).astype(
        np.float32)


def host_prep(inputs):
    xhat = _unit_rows(np.asarray(inputs["x"], np.float32))      # (N, 32)
    tab = np.zeros((N, 64), np.float32)
    tab[:, :F] = xhat

    sels = {d: np.asarray(inputs[f"selected_index_deg{d}"]).astype(np.int64)
            for d in (1, 2, 3, 4)}
    neis = {d: np.asarray(inputs[f"nei_index_deg{d}"]).astype(np.int64)
            .reshape(-1, d) for d in (1, 2, 3, 4)}
    deg = np.zeros(N, np.int8)
    pos = np.zeros(N, np.int64)
    for d in (1, 2, 3, 4):
        deg[sels[d]] = d
        pos[sels[d]] = np.arange(sels[d].shape[0])

    wf_u = {d: _unit_rows(np.asarray(inputs[f"W_focal{d}"], np.float32))
            for d in (1, 2, 3, 4)}                               # (16, 32)
    wn_u = {}
    for d in (1, 2, 3, 4):
        wn = np.asarray(inputs[f"W_nei{d}"], np.float32)         # (16, d, 32)
        wn_u[d] = _unit_rows(wn.reshape(-1, F)).reshape(K, d, F) / d

    # focal stationary [128, 4*64]: per degree d (idx di), 4-block-diag
    wfT = np.zeros((128, 256), np.float32)
    for di, d in enumerate((1, 2, 3, 4)):
        for b in range(4):
            wfT[32 * b:32 * b + 32, 64 * di + 16 * b:64 * di + 16 * b + 16] = \
                wf_u[d].T
    wf_bf = wfT.astype(ml_dtypes.bfloat16)

    in_maps, books = [], []
    for c in range(NCORES):
        lo = c * SHARD
        dc = deg[lo:lo + SHARD]

        # ---- focal: degree-grouped, pre-transposed ----
        xg = np.zeros((SHPAD, F), np.float32)
        fpos = np.zeros(SHARD, np.int64)
        nodes_of_deg = {}
        for di, d in enumerate((1, 2, 3, 4)):
            nd = np.nonzero(dc == d)[0]          # local ids, ascending
            nodes_of_deg[d] = nd
            assert nd.shape[0] <= SEGF[di], (c, d, nd.shape[0])
            o = _SEGB[di]
            xg[o:o + nd.shape[0]] = xhat[lo + nd]
            fpos[nd] = o + np.arange(nd.shape[0])
        xshT = (xg.reshape(SHPAD // 512, 4, 128, F)
                .transpose(1, 3, 0, 2).reshape(128, SHPAD // 4))
        xshT_bf = np.ascontiguousarray(xshT).astype(ml_dtypes.bfloat16)

        # ---- neighbor edge stream ----
        # concat (d,j)-major, node-ascending
        src_l, node_l, dj_l = [], [], []
        for dji, (d, j) in enumerate(DJ):
            nd = nodes_of_deg[d]
            srcs = neis[d][pos[lo + nd], j]
            src_l.append(srcs)
            node_l.append(nd)
            dj_l.append(np.full(nd.shape[0], dji, np.int64))
        src_a = np.concatenate(src_l)
        node_a = np.concatenate(node_l)
        dj_a = np.concatenate(dj_l)
        win_a = src_a // WROWS
        grp = win_a * len(DJ) + dj_a
        order = np.argsort(grp, kind="stable")
        src_s, node_s, dj_s, grp_s = (src_a[order], node_a[order],
                                      dj_a[order], grp[order])

        # per-(win,dj) group sizes -> padded stream positions
        gcnt = np.bincount(grp_s, minlength=NWIN * len(DJ))
        gpad = ((gcnt + 127) // 128) * 128
        wtot = gpad.reshape(NWIN, len(DJ)).sum(1)
        assert wtot.max() <= NIW, wtot.max()
        gbase = np.zeros(NWIN * len(DJ), np.int64)
        for w in range(NWIN):
            b = w * NIW
            for dji in range(len(DJ)):
                g = w * len(DJ) + dji
                gbase[g] = b
                b += gpad[g]
        # dest position of each (sorted) edge
        within = np.arange(src_s.shape[0]) - np.repeat(
            np.cumsum(gcnt) - gcnt, gcnt)
        epos = gbase[grp_s] + within                      # [E] stream slots

        stream_src = np.zeros(ESTREAM, np.int64)          # global row id
        stream_valid = np.zeros(ESTREAM, bool)
        stream_src[epos] = src_s
        stream_valid[epos] = True
        # group pad slots: valid gather (row 0 of own window), zero weights
        colmap = np.full(ESTREAM // 128, -1, np.int64)    # dj code per col
        for w in range(NWIN):
            b = w * NIW
            for dji in range(len(DJ)):
                g = w * len(DJ) + dji
                if gpad[g]:
                    colmap[(gbase[g]) // 128:(gbase[g] + gpad[g]) // 128] = dji
                    sl = slice(gbase[g] + gcnt[g], gbase[g] + gpad[g])
                    stream_src[sl] = w * WROWS
            # tail [b+wtot[w], b+NIW) stays -1 (skipped by num_idxs_reg)

        # chunk counts (min 16 real idxs per chunk)
        cnt_arr = np.zeros((1, NCH), np.int32)
        idx_t = np.zeros((NCH, 128, NI // 16), np.int16)
        for w in range(NWIN):
            for h in range(NCHW):
                ch = w * NCHW + h
                cc = int(min(max(wtot[w] - h * NI, 0), NI))
                if cc < 16:
                    sl = slice(w * NIW + h * NI + cc, w * NIW + h * NI + 16)
                    stream_src[sl] = w * WROWS
                    cc = 16
                cnt_arr[0, ch] = cc
                seg = stream_src[w * NIW + h * NI: w * NIW + (h + 1) * NI]
                iw = (seg - w * WROWS).astype(np.int64)
                kidx = np.where(
                    np.arange(NI) < cc, iw, -1).astype(np.int16)
                idx_t[ch] = np.tile(
                    kidx.reshape(NI // 16, 16).T, (8, 1))

        # per-chunk-col stationaries
        wt4 = np.zeros((NCH, NTILE, 128, 64), np.float32)
        ucat = {dji: wn_u[d][:, j, :] for dji, (d, j) in enumerate(DJ)}
        cm = colmap.reshape(NCH, NI // 128)
        for dji in range(len(DJ)):
            d, j = DJ[dji]
            u = ucat[dji].T                                  # (32, 16)
            chs, cols = np.nonzero(cm == dji)
            t_i, cc_i = cols // 4, cols % 4
            for cc in range(4):
                m = cc_i == cc
                wt4[chs[m], t_i[m], 32 * cc:32 * cc + 32,
                    16 * cc:16 * cc + 16] = u
        wt_arr = (wt4.transpose(0, 2, 1, 3).reshape(NCH, 128, NTILE * 64)
                  .astype(ml_dtypes.bfloat16))

        # assembly bookkeeping per (d,j)
        segbook = []
        for dji in range(len(DJ)):
            m = dj_s == dji
            segbook.append((node_s[m], epos[m]))

        in_maps.append({
            "tab": tab, "idx": idx_t, "cnt": cnt_arr, "wt": wt_arr,
            "xshT": xshT_bf, "wf": wf_bf,
        })
        books.append({"fpos": fpos, "nodes_of_deg": nodes_of_deg,
                      "segbook": segbook})
    return in_maps, books


def assemble(results, books):
    res = np.zeros((N, 64), np.float32)
    for c in range(NCORES):
        lo = c * SHARD
        bk = books[c]
        fo = np.asarray(results[c]["focal_o"]).astype(np.float32)
        focal16 = (fo.reshape(FTILES, 4, 16, 4, 128)
                   .transpose(0, 3, 1, 4, 2).reshape(SHPAD, 16))
        no = np.asarray(results[c]["nei_o"]).astype(np.float32)
        esc = (no.reshape(NCH, 4, 16, NTILE, 128)
               .transpose(0, 3, 1, 4, 2).reshape(ESTREAM, 16))
        nei16 = np.zeros((SHARD, 16), np.float32)
        for nodes_dj, pos_dj in bk["segbook"]:
            nei16[nodes_dj] += esc[pos_dj]
        for di, d in enumerate((1, 2, 3, 4)):
            nd = bk["nodes_of_deg"][d]
            band = slice(16 * (d - 1), 16 * d)
            res[lo + nd, band] = focal16[bk["fpos"][nd]] + nei16[nd]
    return res


def kernel(**inputs):
    global _PROG, LAST_RESULTS
    import os
    from concourse.bass_utils import run_bass_kernel_spmd
    in_maps, books = host_prep(inputs)
    if _PROG is None:
        _PROG = _build_program()
    trace = bool(os.environ.get("BKC_TRACE"))
    res = run_bass_kernel_spmd(_PROG, in_maps, core_ids=list(range(NCORES)),
                               trace=trace)
    LAST_RESULTS = res
    return assemble(res.results, books)


# ---------------------------------------------------------------------------
# numpy emulation of the device program (host-logic validation without HW)
def _emulate_core(m):
    tab = np.asarray(m["tab"])
    out = {}
    bf = lambda a: a.astype(ml_dtypes.bfloat16).astype(np.float32)

    nei = np.zeros((NCH, 64, NTILE * 128), np.float32)
    wt = np.asarray(m["wt"]).astype(np.float32).reshape(NCH, 128, NTILE, 64)
    for ch in range(NCH):
        w = ch // NCHW
        idxs = np.asarray(m["idx"])[ch][:16].T.reshape(-1)[:NI].astype(
            np.int64)
        cc_n = int(np.asarray(m["cnt"])[0, ch])
        g = np.zeros((NI, F), np.float32)
        valid = np.arange(NI) < cc_n
        g[valid] = tab[w * WROWS + idxs[valid], :F]
        gb = bf(g)                                         # device bf16 cast
        g4 = gb.reshape(NTILE, 4, 128, F)
        u4 = np.zeros((NTILE, 4, F, 16), np.float32)
        for t in range(NTILE):
            for cc in range(4):
                u4[t, cc] = wt[ch, 32 * cc:32 * cc + 32, t,
                               16 * cc:16 * cc + 16]
        sc = np.einsum("tcpf,tcfs->tcsp", g4, u4)
        nei[ch] = bf(sc.transpose(0, 1, 2, 3).reshape(
            NTILE, 4 * 16, 128).transpose(1, 0, 2).reshape(64, NTILE * 128))
    out["nei_o"] = nei

    xshT = np.asarray(m["xshT"]).astype(np.float32)        # (128, SHPAD//4)
    wf = np.asarray(m["wf"]).astype(np.float32)            # (128, 256)
    fo = np.zeros((FTILES, 64, 512), np.float32)
    for ft in range(FTILES):
        s = int(np.searchsorted(_SEGB, ft * 2048, side="right")) - 1
        lhsT = wf[:, s * 64:(s + 1) * 64]
        rhs = xshT[:, ft * 512:(ft + 1) * 512]
        fo[ft] = bf(lhsT.T @ rhs)
    out["focal_o"] = fo
    return out


def kernel_emulated(**inputs):
    in_maps, books = host_prep(inputs)
    results = [_emulate_core(m) for m in in_maps]
    return assemble(results, books)
